# revision 1
# baseline (speedup 1.0000x reference)
"""ConvGRUBandCell2d fused Trainium2 kernel (8 NeuronCores, batch-parallel).

v2: 3-stage software pipeline over batches so the PE stream never gaps
(HAM stays warm) and ACT/DVE/GpSimd work overlaps the gate matmuls of the
previous batch.

Reference computation (per pixel (b, f), channels C=512):
  xg = xW @ rmsnorm(x_t; in_w) + xb
  hg = hW @ depthwise_band(rmsnorm(h_prev; hid_w); hmixW, hmixb) + hb
  r = sigmoid(xg_r + hg_r); z = sigmoid(xg_z + hg_z)
  n = tanh(xg_n + r * hg_n)
  h_new = (1 - z) * n + z * h_prev
  out = rmsnorm(h_new + x_t; out_w)

Algebraic refactoring (exact, as v1):
  - in_norm_w folds into xW columns; hid_norm_w folds into the depthwise
    taps; hmixb folds into bh = hW @ hmixb + hb.
  - Per-pixel rms scale commutes with the channel contraction.
  - r/z gates accumulate xW and hW matmul groups in one PSUM chain.

Pipeline stages per batch b (emitted with offsets so engines overlap):
  L(b):  DMA loads of x/h tiles (one iteration ahead)
  F(b):  squares (ACT) -> ssq matmuls (PE) -> sqrt+recip (ACT+DVE)
         -> norm-scale muls (DVE) -> depthwise stencil (DVE)
  B(b):  gate matmuls (PE, r/z chained into [128,1024] PSUM; n-gate
         pairs interleaved with the m-gates) -> sigmoid/tanh (ACT)
         -> h_new (GpSimd sub/mul + DVE adds)
  Y(b):  y squares (ACT) -> ssq matmul (PE) -> sqrt+recip -> out scale
         (DVE) -> DMA out (bf16; host upcasts to f32)

Emission order per iteration i: L(i+1), Y(i-2), F(i), B(i-1).
PE queue: [ynrm(i-2) 8MM][h/x norms(i) 16MM][gates(i-1) 192MM] -- dense.
PSUM: rz pool [128,1024]x2 (4 banks) + chunk pool [128,512]x4 (4 banks).
ACT uses only one table set (sigmoid/tanh/square/copy) plus sqrt, grouped
to 2 table loads per iteration.
"""

import numpy as np

B, C, F, K = 64, 512, 1024, 3
N_CORES = 8
BPC = B // N_CORES          # batches per core
TC = C // 128               # channel tiles (4)
M3 = (3 * C) // 128         # gate-row tiles (12)
EPS = 1e-6

_CACHE = {}


def _build_program():
    import concourse.bacc as bacc
    import concourse.tile as tile
    from concourse import mybir

    f32 = mybir.dt.float32
    bf16 = mybir.dt.bfloat16
    AF = mybir.ActivationFunctionType
    OP = mybir.AluOpType

    nc = bacc.Bacc("TRN2", target_bir_lowering=False, debug=False,
                   num_devices=N_CORES)

    xd = nc.dram_tensor("x", [BPC, C, F], bf16, kind="ExternalInput").ap()
    hd = nc.dram_tensor("h", [BPC, C, F], bf16, kind="ExternalInput").ap()
    xWTd = nc.dram_tensor("xWT", [C, 3 * C], bf16, kind="ExternalInput").ap()
    hWTd = nc.dram_tensor("hWT", [C, 3 * C], bf16, kind="ExternalInput").ap()
    w3d = nc.dram_tensor("w3", [C, K], f32, kind="ExternalInput").ap()
    gbd = nc.dram_tensor("gb", [3 * C, 1], f32, kind="ExternalInput").ap()
    bhnd = nc.dram_tensor("bhn", [C, 1], f32, kind="ExternalInput").ap()
    xbnd = nc.dram_tensor("xbn", [C, 1], f32, kind="ExternalInput").ap()
    wond = nc.dram_tensor("won", [C, 1], f32, kind="ExternalInput").ap()
    onesd = nc.dram_tensor("ones_in", [128, 128], bf16,
                           kind="ExternalInput").ap()
    outd = nc.dram_tensor("out", [BPC, C, F], bf16, kind="ExternalOutput").ap()

    CHS = [slice(0, 512), slice(512, 1024)]

    with tile.TileContext(nc) as tc:
        with (
            tc.tile_pool(name="wp", bufs=1) as wp,
            tc.tile_pool(name="sb", bufs=2) as sb,
            tc.tile_pool(name="prz", bufs=2, space="PSUM") as prz,
            tc.tile_pool(name="pch", bufs=4, space="PSUM") as pch,
        ):
            # ---- resident weights / constants ----
            xw_s, hw_s, w3t = [], [], []
            for k in range(TC):
                xw = wp.tile([128, 3 * C], bf16, tag=f"xw{k}", name=f"xw{k}")
                nc.sync.dma_start(xw[:], xWTd[k * 128:(k + 1) * 128, :])
                xw_s.append(xw)
                hw = wp.tile([128, 3 * C], bf16, tag=f"hw{k}", name=f"hw{k}")
                nc.sync.dma_start(hw[:], hWTd[k * 128:(k + 1) * 128, :])
                hw_s.append(hw)
                w3 = wp.tile([128, K], f32, tag=f"w3{k}", name=f"w3{k}")
                nc.sync.dma_start(w3[:], w3d[k * 128:(k + 1) * 128, :])
                w3t.append(w3)
            ones = wp.tile([128, 128], bf16, tag="ones", name="ones")
            nc.sync.dma_start(ones[:], onesd[:, :])
            gbt = wp.tile([128, M3], f32, tag="gbt", name="gbt")
            nc.sync.dma_start(gbt[:], gbd.rearrange("(m p) o -> p (m o)", p=128))
            bhnt = wp.tile([128, TC], f32, tag="bhnt", name="bhnt")
            nc.sync.dma_start(bhnt[:], bhnd.rearrange("(m p) o -> p (m o)", p=128))
            xbnt = wp.tile([128, TC], f32, tag="xbnt", name="xbnt")
            nc.sync.dma_start(xbnt[:], xbnd.rearrange("(m p) o -> p (m o)", p=128))
            wont = wp.tile([128, TC], f32, tag="wont", name="wont")
            nc.sync.dma_start(wont[:], wond.rearrange("(m p) o -> p (m o)", p=128))
            epst = wp.tile([128, 1], f32, tag="epst", name="epst")
            nc.vector.memset(epst[:], EPS)

            onb = ones[:]
            st = [dict() for _ in range(BPC)]

            def stage_L(b):
                s = st[b]
                s["ht"] = []
                s["xt"] = []
                for ct in range(TC):
                    t = sb.tile([128, F], bf16, tag=f"ht{ct}", name=f"ht{b}_{ct}")
                    nc.sync.dma_start(t[:], hd[b, ct * 128:(ct + 1) * 128, :])
                    s["ht"].append(t)
                for ct in range(TC):
                    t = sb.tile([128, F], bf16, tag=f"xt{ct}", name=f"xt{b}_{ct}")
                    nc.sync.dma_start(t[:], xd[b, ct * 128:(ct + 1) * 128, :])
                    s["xt"].append(t)

            def norm_chain(src_tiles, nm, b):
                """squares (ACT) -> ssq MMs (PE, 2 chunk tiles) -> sqrt (ACT)
                -> recip (DVE) -> returns inv [128,F] f32 tile."""
                sqs = []
                for ct in range(TC):
                    q = sb.tile([128, F], bf16, tag=f"sq{ct}", bufs=2,
                                name=f"sq{nm}{b}_{ct}")
                    nc.scalar.square(q[:], src_tiles[ct][:])
                    sqs.append(q)
                pts = []
                for ch in range(2):
                    p = pch.tile([128, 512], f32, tag="chk", bufs=4,
                                 name=f"n{nm}{b}_{ch}")
                    for ct in range(TC):
                        nc.tensor.matmul(p[:], onb, sqs[ct][:, CHS[ch]],
                                         start=(ct == 0), stop=(ct == TC - 1))
                    pts.append(p)
                sr = sb.tile([128, F], f32, tag="sr", bufs=2, name=f"sr{nm}{b}")
                for ch in range(2):
                    nc.scalar.activation(sr[:, CHS[ch]], pts[ch][:], AF.Sqrt,
                                         bias=epst[:], scale=1.0 / C)
                inv = sb.tile([128, F], f32, tag=f"inv{nm}", bufs=1,
                              name=f"inv{nm}{b}")
                nc.vector.reciprocal_approx_fast(inv[:], sr[:])
                return inv

            def stage_F(b):
                s = st[b]
                invh = norm_chain(s["ht"], "h", b)
                invx = norm_chain(s["xt"], "x", b)
                s["hs"] = []
                s["xs"] = []
                s["hm"] = []
                for ct in range(TC):
                    t = sb.tile([128, F + 2], bf16, tag=f"hs{ct}", bufs=1,
                                name=f"hs{b}_{ct}")
                    if b == 0:
                        nc.vector.memset(t[:, 0:1], 0.0)
                        nc.vector.memset(t[:, F + 1:F + 2], 0.0)
                    nc.vector.tensor_mul(t[:, 1:F + 1], s["ht"][ct][:], invh[:])
                    s["hs"].append(t)
                for ct in range(TC):
                    t = sb.tile([128, F], bf16, tag=f"xs{ct}", name=f"xs{b}_{ct}")
                    nc.vector.tensor_mul(t[:], s["xt"][ct][:], invx[:])
                    s["xs"].append(t)
                for ct in range(TC):
                    hs = s["hs"][ct]
                    t = sb.tile([128, F], bf16, tag=f"hm{ct}", name=f"hm{b}_{ct}")
                    nc.vector.tensor_scalar_mul(t[:], hs[:, 1:F + 1],
                                                w3t[ct][:, 1:2])
                    nc.vector.scalar_tensor_tensor(
                        t[:], hs[:, 0:F], w3t[ct][:, 0:1], t[:],
                        OP.mult, OP.add)
                    nc.vector.scalar_tensor_tensor(
                        t[:], hs[:, 2:F + 2], w3t[ct][:, 2:3], t[:],
                        OP.mult, OP.add)
                    s["hm"].append(t)

            def emit_rz_gate(b, m):
                """One [128,1024] PSUM chain: 4 xw + 4 hw matmuls per chunk,
                then one sigmoid over both chunks."""
                s = st[b]
                ps = prz.tile([128, F], f32, tag="rz", name=f"rz{b}_{m}")
                for k in range(TC):
                    w = xw_s[k][:, m * 128:(m + 1) * 128]
                    for ch in range(2):
                        nc.tensor.matmul(ps[:, CHS[ch]], w,
                                         s["xs"][k][:, CHS[ch]],
                                         start=(k == 0), stop=False)
                for k in range(TC):
                    w = hw_s[k][:, m * 128:(m + 1) * 128]
                    for ch in range(2):
                        nc.tensor.matmul(ps[:, CHS[ch]], w,
                                         s["hm"][k][:, CHS[ch]],
                                         start=False, stop=(k == TC - 1))
                if m < 4:
                    g = sb.tile([128, F], bf16, tag=f"rg{m}", bufs=1,
                                name=f"rg{b}_{m}")
                    s["rg"].append(g)
                else:
                    g = s["ug"][m - 4]
                nc.scalar.activation(g[:], ps[:], AF.Sigmoid,
                                     bias=gbt[:, m:m + 1])

            def emit_n_round(b, j, ch):
                s = st[b]
                S = CHS[ch]
                m = 8 + j
                psx = pch.tile([128, 512], f32, tag="chk",
                               name=f"npx{b}_{j}_{ch}")
                for k in range(TC):
                    nc.tensor.matmul(psx[:], xw_s[k][:, m * 128:(m + 1) * 128],
                                     s["xs"][k][:, S],
                                     start=(k == 0), stop=(k == TC - 1))
                psh = pch.tile([128, 512], f32, tag="chk",
                               name=f"nph{b}_{j}_{ch}")
                for k in range(TC):
                    nc.tensor.matmul(psh[:], hw_s[k][:, m * 128:(m + 1) * 128],
                                     s["hm"][k][:, S],
                                     start=(k == 0), stop=(k == TC - 1))
                t = sb.tile([128, 512], bf16, tag="nt", bufs=3,
                            name=f"nt{b}_{j}_{ch}")
                nc.vector.scalar_tensor_tensor(
                    t[:], psh[:], bhnt[:, j:j + 1], s["rg"][j][:, S],
                    OP.add, OP.mult)
                nc.vector.tensor_add(t[:], t[:], psx[:])
                nc.scalar.activation(s["cg"][j][:, S], t[:], AF.Tanh,
                                     bias=xbnt[:, j:j + 1])

            def stage_B(b):
                s = st[b]
                s["rg"] = []
                s["ug"] = [sb.tile([128, F], bf16, tag=f"ug{j}", bufs=1,
                                   name=f"ug{b}_{j}") for j in range(4)]
                s["cg"] = [sb.tile([128, F], bf16, tag=f"cg{j}", bufs=1,
                                   name=f"cg{b}_{j}") for j in range(4)]
                # r gates first (n rounds need them), n rounds interleaved
                # with the remaining m gates to hide the DVE psum reads.
                for m in range(5):
                    emit_rz_gate(b, m)
                nseq = [(0, 0), (0, 1), (1, 0), (1, 1), (2, 0), (2, 1),
                        (3, 0), (3, 1)]
                for j, ch in nseq[:2]:
                    emit_n_round(b, j, ch)
                emit_rz_gate(b, 5)
                for j, ch in nseq[2:4]:
                    emit_n_round(b, j, ch)
                emit_rz_gate(b, 6)
                for j, ch in nseq[4:6]:
                    emit_n_round(b, j, ch)
                emit_rz_gate(b, 7)
                for j, ch in nseq[6:]:
                    emit_n_round(b, j, ch)
                # h_new + x_t  (adds on GpSimd to offload DVE; squares here so
                # the y-norm matmuls at the next iteration boundary never wait
                # on the ACT queue)
                s["y"] = []
                for ct in range(TC):
                    y = sb.tile([128, F], bf16, tag=f"yt{ct}", name=f"yt{b}_{ct}")
                    nc.gpsimd.tensor_sub(y[:], s["ht"][ct][:], s["cg"][ct][:])
                    nc.gpsimd.tensor_mul(y[:], y[:], s["ug"][ct][:])
                    nc.vector.tensor_add(y[:], y[:], s["cg"][ct][:])
                    nc.vector.tensor_add(y[:], y[:], s["xt"][ct][:])
                    s["y"].append(y)

            def stage_Y(b):
                s = st[b]
                pts = [pch.tile([128, 512], f32, tag="chk", name=f"yn{b}_{ch}")
                       for ch in range(2)]
                for ct in range(TC):
                    q = sb.tile([128, F], bf16, tag="ysq", bufs=2,
                                name=f"ysq{b}_{ct}")
                    nc.scalar.square(q[:], s["y"][ct][:])
                    for ch in range(2):
                        nc.tensor.matmul(pts[ch][:], onb, q[:, CHS[ch]],
                                         start=(ct == 0), stop=(ct == TC - 1))
                sr = sb.tile([128, F], f32, tag="sr", bufs=2, name=f"sry{b}")
                for ch in range(2):
                    nc.scalar.activation(sr[:, CHS[ch]], pts[ch][:], AF.Sqrt,
                                         bias=epst[:], scale=1.0 / C)
                invy = sb.tile([128, F], f32, tag="invy", bufs=1,
                               name=f"invy{b}")
                nc.vector.reciprocal_approx_fast(invy[:], sr[:])
                for ct in range(TC):
                    o = sb.tile([128, F], bf16, tag="ot", bufs=3,
                                name=f"ot{b}_{ct}")
                    nc.vector.scalar_tensor_tensor(
                        o[:], s["y"][ct][:], wont[:, ct:ct + 1], invy[:],
                        OP.mult, OP.mult)
                    nc.sync.dma_start(
                        outd[b, ct * 128:(ct + 1) * 128, :], o[:])
                st[b] = {}

            stage_L(0)
            for i in range(BPC + 2):
                if i + 1 < BPC:
                    stage_L(i + 1)
                if i >= 2:
                    stage_Y(i - 2)
                if i < BPC:
                    stage_F(i)
                if 1 <= i <= BPC:
                    stage_B(i - 1)

    nc.compile()
    return nc


def _get_program():
    if "nc" not in _CACHE:
        _CACHE["nc"] = _build_program()
    return _CACHE["nc"]


def kernel(x_t, h_prev, in_norm_w, hid_norm_w, out_norm_w,
           xW, xb, hmixW, hmixb, hW, hb):
    import ml_dtypes
    from concourse.bass_utils import run_bass_kernel_spmd

    nc = _get_program()

    f = np.float32
    b16 = ml_dtypes.bfloat16
    x = np.ascontiguousarray(np.asarray(x_t, f).reshape(B, C, F).astype(b16))
    h = np.ascontiguousarray(np.asarray(h_prev, f).reshape(B, C, F).astype(b16))
    xW = np.asarray(xW, f)
    hW = np.asarray(hW, f)
    xWT = np.ascontiguousarray(
        (xW * np.asarray(in_norm_w, f)[None, :]).T.astype(b16))
    hWT = np.ascontiguousarray(hW.T.astype(b16))
    w3 = np.ascontiguousarray(
        np.asarray(hmixW, f)[:, 0, 0, :] * np.asarray(hid_norm_w, f)[:, None])
    bh = hW @ np.asarray(hmixb, f) + np.asarray(hb, f)
    gb = np.ascontiguousarray((np.asarray(xb, f) + bh).reshape(3 * C, 1))
    bhn = np.ascontiguousarray(bh[2 * C:].reshape(C, 1))
    xbn = np.ascontiguousarray(np.asarray(xb, f)[2 * C:].reshape(C, 1))
    won = np.ascontiguousarray(np.asarray(out_norm_w, f).reshape(C, 1))

    shared = {"xWT": xWT, "hWT": hWT, "w3": w3, "gb": gb, "bhn": bhn,
              "xbn": xbn, "won": won,
              "ones_in": np.ones((128, 128), dtype=b16)}
    in_maps = []
    for c in range(N_CORES):
        m = dict(shared)
        m["x"] = x[c * BPC:(c + 1) * BPC]
        m["h"] = h[c * BPC:(c + 1) * BPC]
        in_maps.append(m)

    res = run_bass_kernel_spmd(nc, in_maps, core_ids=list(range(N_CORES)),
                               **_CACHE.get("run_kwargs", {}))
    _CACHE["last_results"] = res
    out = np.concatenate([res.results[c]["out"] for c in range(N_CORES)],
                         axis=0)
    return out.reshape(B, C, 1, F).astype(np.float32)

